# revision 12
# baseline (speedup 1.0000x reference)
"""MoE layer (T=8192, d=1024, dff=1024, E=64, top-k=2, capacity factor 2)
on 8 Trainium2 NeuronCores, expert-parallel.

Strategy
--------
Host (cheap, O(N) index math + gathers):
  * compute each expanded token's expert and its position within the expert
    (the reference's cumsum-over-one-hot routing), applying the capacity cap
  * experts are sharded 8-per-core; tokens routed to an expert are packed
    into a [d, cols] column block for that expert, TRANSPOSED and cast to
    bf16 so the device needs no on-chip transposes
  * every DRAM tensor is repacked PARTITION-MAJOR on the host so each
    device DMA is 128 descriptors of >=4KB contiguous runs (full HBM rate,
    ~0.6us issue cost on the DGE queue instead of 4us+ for the strided
    layout)
  * per-expert column counts are padded to the max across cores so all 8
    cores run one identical SPMD program (shapes baked at build time)

Device (one Bass/Tile program, built for the observed count vector):
  * per expert e: h_T = W1_e^T @ x_T  (PSUM, fp32 accum)
                  act_T = silu(gate_T) * up_T   (ACT + DVE, cast bf16)
                  y_T  = W2_e^T @ act_T          (PSUM, fp32 accum)
  * weights stream HBM->SBUF double-buffered on the sync-engine HWDGE ring
    in exactly PE consumption order; token blocks stream on the gpsimd
    ring so they never delay the weight stream; y stores go out on the
    scalar ring
  * warm-up: PE-clock ramp is bridged with junk matmuls emitted as
    accumulation groups over two alternating PSUM banks (accumulating
    matmuls pipeline back-to-back; independent start/stop matmuls onto one
    bank serialize on PSUM write-back and would block the real stream)
  * expert 0's x/w1 arrive in per-k-tile chunks and are processed k-outer
    so its matmuls trickle-start as soon as the first chunks land
  * the last expert's y store is split per oi-pair so the final wire+
    semaphore latency that gates the framework teardown is ~0.5us, not 2.5

Host combine: gather each expanded token's output column, weight by router
prob, sum over the k=2 copies.
"""

import numpy as np
import ml_dtypes

from concourse import bacc, mybir
import concourse.tile as tile
from concourse.bass_utils import run_bass_kernel_spmd

P = 128
NCORES = 8

BF16 = ml_dtypes.bfloat16

_program_cache: dict = {}


def _build_program(cnts: tuple, d: int, dff: int, epc: int):
    """Build+compile the SPMD Bass program for per-slot column counts `cnts`."""
    ctot = sum(cnts)
    f32 = mybir.dt.float32
    bf = mybir.dt.bfloat16

    fp8 = mybir.dt.float8e3

    nc = bacc.Bacc("TRN2", target_bir_lowering=False, debug=False)
    # All tensors partition-major: [P, ...] with per-partition bytes
    # contiguous, so every DMA is 128 long descriptors.
    xp = nc.declare_dram_parameter("xp", [P, (d // P) * ctot], bf, isOutput=False)
    # w1 streams as fp8 e3m4 scaled by 64 (exact power of 2): halves the
    # weight bytes on the critical expert-0 window. The PE runs the mixed
    # fp8xbf16 matmul at bf16 speed (cost keys on the moving operand); the
    # x64 is divided out of gate exactly by the silu's input scale, rides
    # the up-half linearly through act/mm2/y, and the host combine divides
    # it out of the output. w2 stays bf16 (w1+w2 both fp8 would breach the
    # 2e-2 gate: 2.1% vs 1.7% measured).
    gup = nc.declare_dram_parameter("gup", [epc, P, (d // P) * 2 * dff], fp8,
                                    isOutput=False)
    dn = nc.declare_dram_parameter("dn", [epc, P, (dff // P) * d], bf,
                                   isOutput=False)
    y = nc.declare_dram_parameter("y", [P, (d // P) * ctot], bf, isOutput=True)

    OT = d // P
    KT = d // P    # contraction tiles for mm1
    FT = dff // P  # dff tiles (rows of h_T per gate/up half)
    nmax = max(cnts)

    offs = []
    acc = 0
    for j in range(epc):
        offs.append(acc)
        acc += cnts[j]

    with tile.TileContext(nc) as tc:
        with (
            tc.tile_pool(name="xpool", bufs=1) as xpool,
            tc.tile_pool(name="w1pool", bufs=2) as w1pool,
            tc.tile_pool(name="w2pool", bufs=2) as w2pool,
            tc.tile_pool(name="actpool", bufs=2) as actpool,
            tc.tile_pool(name="ypool", bufs=3) as ypool,
            tc.tile_pool(name="evict", bufs=3) as evict,
        ):
            # x stays resident; expert j's block is [P, KT*nj] contiguous
            xt = xpool.tile([P, KT * ctot], bf, tag="xt")
            scratch = evict.tile([P, nmax], bf, tag="scratch")
            nc.gpsimd.memset(scratch[:], 0.0)

            nj0 = cnts[0]
            w1_0 = w1pool.tile([P, KT, 2 * dff], fp8, tag="w1")
            g3_0 = gup[0].rearrange("p (kk c) -> p kk c", kk=KT)
            act_0 = actpool.tile([P, FT, nmax], bf, tag="act")

            # Expert 0's mm1 runs in a dedicated 8-bank PSUM scope: full
            # k-outer order with 8 open accumulation groups (gate/up x 4
            # i-blocks per half), so every arriving x/w1 k-chunk enables 16
            # matmuls instead of 4 — the DMA-led warm-up window stays busy.
            with tc.tile_pool(name="ps0", bufs=1, space="PSUM") as ps0:
                # PE clock-gate warm-up: accumulation groups pipeline with
                # no PSUM write-back stalls; alternate banks so group N+1's
                # start doesn't wait on group N's retire.
                for g in range(5):
                    jp = ps0.tile([P, nj0], f32, tag=f"a{g % 2}")
                    for a in range(4):
                        nc.tensor.matmul(
                            jp[:], lhsT=scratch[:, :P], rhs=scratch[:, :nj0],
                            start=(a == 0), stop=(a == 3),
                        )
                # x0 split across the sync+scalar rings (two DMA queues
                # drain concurrently, and phase A's kk=0-3 matmuls only
                # depend on the first half); w1_0 chunked per k-tile so the
                # k-outer matmuls trickle-start chunk by chunk behind it
                h0 = (KT // 2) * nj0
                nc.sync.dma_start(xt[:, :h0], xp[:, :h0])
                nc.scalar.dma_start(
                    xt[:, h0 : KT * nj0], xp[:, h0 : KT * nj0]
                )
                for kk in range(KT):
                    nc.sync.dma_start(w1_0[:, kk, :], g3_0[:, kk, :])
                for half in range(2):
                    accs = [
                        ps0.tile([P, nj0], f32, tag=f"a{t}", name=f"acc{t}")
                        for t in range(8)
                    ]
                    for kk in range(KT):
                        x_ap = xt[:, kk * nj0 : kk * nj0 + nj0]
                        for i in range(4):
                            ii = half * 4 + i
                            nc.tensor.matmul(
                                accs[2 * i][:],
                                lhsT=w1_0[:, kk, ii * P : (ii + 1) * P],
                                rhs=x_ap, start=(kk == 0), stop=(kk == KT - 1),
                            )
                            nc.tensor.matmul(
                                accs[2 * i + 1][:],
                                lhsT=w1_0[:, kk, dff + ii * P : dff + (ii + 1) * P],
                                rhs=x_ap, start=(kk == 0), stop=(kk == KT - 1),
                            )
                    for i in range(4):
                        ii = half * 4 + i
                        silu_sb = evict.tile([P, nj0], f32, tag="silu")
                        nc.scalar.activation(
                            silu_sb[:], accs[2 * i][:],
                            mybir.ActivationFunctionType.Silu, scale=1.0 / 64,
                        )
                        nc.vector.tensor_mul(
                            act_0[:, ii, :nj0], silu_sb[:], accs[2 * i + 1][:]
                        )

            with (
                tc.tile_pool(name="ps1", bufs=2, space="PSUM") as ps1,
                tc.tile_pool(name="ps2", bufs=2, space="PSUM") as ps2,
            ):
                def mm1_swiglu(w1, act, base, nj, i):
                    gate_ps = ps1.tile([P, nj], f32, tag="gate")
                    up_ps = ps1.tile([P, nj], f32, tag="up")
                    for kk in range(KT):
                        nc.tensor.matmul(
                            gate_ps[:],
                            lhsT=w1[:, kk, i * P : (i + 1) * P],
                            rhs=xt[:, base + kk * nj : base + kk * nj + nj],
                            start=(kk == 0),
                            stop=(kk == KT - 1),
                        )
                    for kk in range(KT):
                        nc.tensor.matmul(
                            up_ps[:],
                            lhsT=w1[:, kk, dff + i * P : dff + (i + 1) * P],
                            rhs=xt[:, base + kk * nj : base + kk * nj + nj],
                            start=(kk == 0),
                            stop=(kk == KT - 1),
                        )
                    silu_sb = evict.tile([P, nj], f32, tag="silu")
                    nc.scalar.activation(
                        silu_sb[:], gate_ps[:],
                        mybir.ActivationFunctionType.Silu, scale=1.0 / 64,
                    )
                    nc.vector.tensor_mul(act[:, i, :nj], silu_sb[:], up_ps[:])

                def mm2_store(w2, act, j, nj):
                    yt = ypool.tile([P, OT * nj], bf, tag="yt")
                    ybase = OT * offs[j]
                    last = j == epc - 1
                    for oi in range(OT):
                        y_ps = ps2.tile([P, nj], f32, tag="y")
                        for kk in range(FT):
                            nc.tensor.matmul(
                                y_ps[:],
                                lhsT=w2[:, kk, oi * P : (oi + 1) * P],
                                rhs=act[:, kk, :nj],
                                start=(kk == 0),
                                stop=(kk == FT - 1),
                            )
                        nc.vector.tensor_copy(
                            yt[:, oi * nj : (oi + 1) * nj], y_ps[:]
                        )
                        if last and (oi % 2 == 1 or oi >= OT - 2):
                            # stream the tail out per oi-pair (singles for
                            # the final two): the last chunk's wire+sem
                            # latency gates the framework teardown
                            lo = oi - 1 if (oi % 2 == 1 and oi < OT - 2) else oi
                            nc.scalar.dma_start(
                                y[:, ybase + lo * nj : ybase + (oi + 1) * nj],
                                yt[:, lo * nj : (oi + 1) * nj],
                            )
                    if not last:
                        # scalar ring so stores never delay the weight stream
                        nc.scalar.dma_start(y[:, ybase : ybase + OT * nj], yt[:])

                # Two overlap tricks at each expert boundary:
                # * expert j's first two mm1 i-blocks are emitted BEFORE
                #   expert j-1's mm2, so the PE fills the window where
                #   act_{j-1}'s last silu is still in flight
                # * w2_{j-1}'s DMA is issued AFTER x_j/w1_j on the ring:
                #   mm1_j[i0] needs w1_j a few us before mm2_{j-1} needs
                #   w2_{j-1}, so the wire delivers in consumption order
                act_prev, j_prev, nj_prev = act_0, 0, nj0
                for j in range(1, epc):
                    nj = cnts[j]
                    base = KT * offs[j]
                    nc.sync.dma_start(
                        xt[:, base : base + KT * nj],
                        xp[:, base : base + KT * nj],
                    )
                    w1 = w1pool.tile([P, KT, 2 * dff], fp8, tag="w1")
                    nc.sync.dma_start(
                        w1[:], gup[j].rearrange("p (kk c) -> p kk c", kk=KT)
                    )
                    w2p = w2pool.tile([P, FT, d], bf, tag="w2")
                    nc.sync.dma_start(
                        w2p[:], dn[j_prev].rearrange("p (kk c) -> p kk c", kk=FT)
                    )
                    act = actpool.tile([P, FT, nmax], bf, tag="act")
                    for i in range(2):
                        mm1_swiglu(w1, act, base, nj, i)
                    mm2_store(w2p, act_prev, j_prev, nj_prev)
                    for i in range(2, FT):
                        mm1_swiglu(w1, act, base, nj, i)
                    act_prev, j_prev, nj_prev = act, j, nj
                w2l = w2pool.tile([P, FT, d], bf, tag="w2")
                nc.sync.dma_start(
                    w2l[:], dn[j_prev].rearrange("p (kk c) -> p kk c", kk=FT)
                )
                mm2_store(w2l, act_prev, j_prev, nj_prev)

    nc.compile()
    return nc


def _route(topk_indices: np.ndarray, E: int, C: int):
    """Reference-equivalent routing: per expanded token, its within-expert
    position in flat (t, k) order; tokens beyond capacity C are dropped."""
    e = np.asarray(topk_indices).reshape(-1).astype(np.int64)
    N = e.shape[0]
    order = np.argsort(e, kind="stable")  # grouped by expert, flat order kept
    counts = np.bincount(e, minlength=E)
    starts = np.zeros(E + 1, np.int64)
    np.cumsum(counts, out=starts[1:])
    rank = np.arange(N, dtype=np.int64) - starts[e[order]]  # pos within expert
    pos = np.empty(N, np.int64)
    pos[order] = rank
    keep = pos < C
    return e, pos, keep, counts


def kernel(
    hidden_states: np.ndarray,
    topk_indices: np.ndarray,
    topk_weights: np.ndarray,
    gate_up_proj: np.ndarray,
    down_proj: np.ndarray,
) -> np.ndarray:
    hs = np.asarray(hidden_states, dtype=np.float32)
    tw = np.asarray(topk_weights, dtype=np.float32)
    gupw = np.asarray(gate_up_proj, dtype=np.float32)
    dnw = np.asarray(down_proj, dtype=np.float32)

    T, d = hs.shape
    k = np.asarray(topk_indices).shape[-1]
    E, _, dff2 = gupw.shape
    dff = dff2 // 2
    N = T * k
    C = (2 * N) // E  # CAPACITY_FACTOR = 2
    epc = E // NCORES
    KT = d // P
    OT = d // P

    e, pos, keep, _ = _route(topk_indices, E, C)
    posc = np.minimum(pos, C - 1)
    kept_idx = np.where(keep)[0]

    # Kept-token count per expert, then deal experts to (slot, core) by global
    # rank: slot j on every core holds the experts ranked 8j..8j+7 by count.
    # Per-slot counts are then nearly equal across cores, so the SPMD padding
    # (max over cores) wastes ~2% instead of ~10%. Slot 0 is the biggest
    # (overlaps the startup DMA trickle), the last slot smallest (short tail).
    ce = np.bincount(e[kept_idx], minlength=E)
    order = np.argsort(-ce, kind="stable")  # experts by count, descending
    assign = order.reshape(epc, NCORES)  # [slot j, core m] -> expert id
    core_of_expert = np.empty(E, np.int64)
    slot_of_expert = np.empty(E, np.int64)
    for j in range(epc):
        for m in range(NCORES):
            core_of_expert[assign[j, m]] = m
            slot_of_expert[assign[j, m]] = j

    cnts = tuple(
        int(max(int(ce[assign[j]].max()), 1)) for j in range(epc)
    )
    ctot = sum(cnts)
    offs_prog = np.zeros(epc, np.int64)
    np.cumsum(np.asarray(cnts[:-1], np.int64), out=offs_prog[1:])

    core_of = core_of_expert[e]  # per expanded token
    # column of each kept expanded token inside its core's packed layout
    col = offs_prog[slot_of_expert[e]] + posc  # valid where keep

    key = (cnts, d, dff, epc)
    nc = _program_cache.get(key)
    if nc is None:
        nc = _build_program(cnts, d, dff, epc)
        _program_cache[key] = nc

    hsb = hs.astype(BF16)
    tok_of_n = np.arange(N, dtype=np.int64) // k

    # Weights partition-major: [P, KT*cols] with per-partition contiguous
    # bytes (row d-index kk*128+p -> [p, kk*cols:(kk+1)*cols]).
    gup_r = np.ascontiguousarray(
        (gupw * 64.0).reshape(E, KT, P, 2 * dff).transpose(0, 2, 1, 3)
    ).reshape(E, P, KT * 2 * dff).astype(ml_dtypes.float8_e3m4)
    dn_r = np.ascontiguousarray(
        dnw.reshape(E, dff // P, P, d).transpose(0, 2, 1, 3)
    ).reshape(E, P, (dff // P) * d).astype(BF16)

    in_maps = []
    for m in range(NCORES):
        X = np.zeros((ctot, d), BF16)
        sel = kept_idx[core_of[kept_idx] == m]
        X[col[sel]] = hsb[tok_of_n[sel]]
        xT4 = X.T.reshape(KT, P, ctot)  # [kk, p, col]
        xp = np.empty((P, KT * ctot), BF16)
        for j in range(epc):
            off, nj = int(offs_prog[j]), cnts[j]
            xp[:, KT * off : KT * (off + nj)] = (
                xT4[:, :, off : off + nj].transpose(1, 0, 2).reshape(P, KT * nj)
            )
        eids = assign[:, m]  # this core's experts in program (slot) order
        in_maps.append(
            {
                "xp": xp,
                "gup": np.ascontiguousarray(gup_r[eids]),
                "dn": np.ascontiguousarray(dn_r[eids]),
            }
        )

    res = run_bass_kernel_spmd(nc, in_maps, list(range(NCORES)))

    # combine: rows[n] = y_core(n)[:, col(n)] for kept n, 0 otherwise
    rows = np.zeros((N, d), np.float32)
    for m in range(NCORES):
        Yp = np.asarray(res.results[m]["y"])  # [P, OT*ctot] bf16 packed
        Y = np.empty((d, ctot), np.float32)
        for j in range(epc):
            off, nj = int(offs_prog[j]), cnts[j]
            Y[:, off : off + nj] = (
                Yp[:, OT * off : OT * (off + nj)]
                .reshape(P, OT, nj).transpose(1, 0, 2).reshape(d, nj)
            )
        sel = kept_idx[core_of[kept_idx] == m]
        rows[sel] = Y.T[col[sel]]
    wf = tw.reshape(-1) * keep.astype(np.float32) * (1.0 / 64.0)
    out = (rows * wf[:, None]).reshape(T, k, d).sum(axis=1)
    return out.astype(hs.dtype)


# revision 13
# speedup vs baseline: 1.0010x; 1.0010x over previous
"""MoE layer (T=8192, d=1024, dff=1024, E=64, top-k=2, capacity factor 2)
on 8 Trainium2 NeuronCores, expert-parallel.

Strategy
--------
Host (cheap, O(N) index math + gathers):
  * compute each expanded token's expert and its position within the expert
    (the reference's cumsum-over-one-hot routing), applying the capacity cap
  * experts are sharded 8-per-core; tokens routed to an expert are packed
    into a [d, cols] column block for that expert, TRANSPOSED and cast to
    bf16 so the device needs no on-chip transposes
  * every DRAM tensor is repacked PARTITION-MAJOR on the host so each
    device DMA is 128 descriptors of >=4KB contiguous runs (full HBM rate,
    ~0.6us issue cost on the DGE queue instead of 4us+ for the strided
    layout)
  * per-expert column counts are padded to the max across cores so all 8
    cores run one identical SPMD program (shapes baked at build time)

Device (one Bass/Tile program, built for the observed count vector):
  * per expert e: h_T = W1_e^T @ x_T  (PSUM, fp32 accum)
                  act_T = silu(gate_T) * up_T   (ACT + DVE, cast bf16)
                  y_T  = W2_e^T @ act_T          (PSUM, fp32 accum)
  * weights stream HBM->SBUF double-buffered on the sync-engine HWDGE ring
    in exactly PE consumption order; token blocks stream on the gpsimd
    ring so they never delay the weight stream; y stores go out on the
    scalar ring
  * warm-up: PE-clock ramp is bridged with junk matmuls emitted as
    accumulation groups over two alternating PSUM banks (accumulating
    matmuls pipeline back-to-back; independent start/stop matmuls onto one
    bank serialize on PSUM write-back and would block the real stream)
  * expert 0's x/w1 arrive in per-k-tile chunks and are processed k-outer
    so its matmuls trickle-start as soon as the first chunks land
  * the last expert's y store is split per oi-pair so the final wire+
    semaphore latency that gates the framework teardown is ~0.5us, not 2.5

Host combine: gather each expanded token's output column, weight by router
prob, sum over the k=2 copies.
"""

import numpy as np
import ml_dtypes

from concourse import bacc, mybir
import concourse.tile as tile
from concourse.bass_utils import run_bass_kernel_spmd

P = 128
NCORES = 8

BF16 = ml_dtypes.bfloat16

_program_cache: dict = {}


def _build_program(cnts: tuple, d: int, dff: int, epc: int):
    """Build+compile the SPMD Bass program for per-slot column counts `cnts`."""
    ctot = sum(cnts)
    f32 = mybir.dt.float32
    bf = mybir.dt.bfloat16

    fp8 = mybir.dt.float8e3

    nc = bacc.Bacc("TRN2", target_bir_lowering=False, debug=False)
    # All tensors partition-major: [P, ...] with per-partition bytes
    # contiguous, so every DMA is 128 long descriptors.
    xp = nc.declare_dram_parameter("xp", [P, (d // P) * ctot], bf, isOutput=False)
    # w1 streams as fp8 e3m4 scaled by 64 (exact power of 2): halves the
    # weight bytes on the critical expert-0 window. The PE runs the mixed
    # fp8xbf16 matmul at bf16 speed (cost keys on the moving operand); the
    # x64 is divided out of gate exactly by the silu's input scale, rides
    # the up-half linearly through act/mm2/y, and the host combine divides
    # it out of the output. w2 stays bf16 (w1+w2 both fp8 would breach the
    # 2e-2 gate: 2.1% vs 1.7% measured).
    gup = nc.declare_dram_parameter("gup", [epc, P, (d // P) * 2 * dff], fp8,
                                    isOutput=False)
    dn = nc.declare_dram_parameter("dn", [epc, P, (dff // P) * d], bf,
                                   isOutput=False)
    y = nc.declare_dram_parameter("y", [P, (d // P) * ctot], bf, isOutput=True)

    OT = d // P
    KT = d // P    # contraction tiles for mm1
    FT = dff // P  # dff tiles (rows of h_T per gate/up half)
    nmax = max(cnts)

    offs = []
    acc = 0
    for j in range(epc):
        offs.append(acc)
        acc += cnts[j]

    with tile.TileContext(nc) as tc:
        with (
            tc.tile_pool(name="xpool", bufs=1) as xpool,
            tc.tile_pool(name="w1pool", bufs=2) as w1pool,
            tc.tile_pool(name="w2pool", bufs=2) as w2pool,
            tc.tile_pool(name="actpool", bufs=2) as actpool,
            tc.tile_pool(name="ypool", bufs=3) as ypool,
            tc.tile_pool(name="evict", bufs=3) as evict,
        ):
            # x stays resident; expert j's block is [P, KT*nj] contiguous
            xt = xpool.tile([P, KT * ctot], bf, tag="xt")
            scratch = evict.tile([P, nmax], bf, tag="scratch")
            nc.gpsimd.memset(scratch[:], 0.0)

            nj0 = cnts[0]
            w1_0 = w1pool.tile([P, KT, 2 * dff], fp8, tag="w1")
            g3_0 = gup[0].rearrange("p (kk c) -> p kk c", kk=KT)
            w2_00 = w2pool.tile([P, FT, d], bf, tag="w2")
            act_0 = actpool.tile([P, FT, nmax], bf, tag="act")

            # Expert 0's mm1 runs in a dedicated 8-bank PSUM scope: full
            # k-outer order with 8 open accumulation groups (gate/up x 4
            # i-blocks per half), so every arriving x/w1 k-chunk enables 16
            # matmuls instead of 4 — the DMA-led warm-up window stays busy.
            with tc.tile_pool(name="ps0", bufs=1, space="PSUM") as ps0:
                # PE clock-gate warm-up: accumulation groups pipeline with
                # no PSUM write-back stalls; alternate banks so group N+1's
                # start doesn't wait on group N's retire.
                for g in range(5):
                    jp = ps0.tile([P, nj0], f32, tag=f"a{g % 2}")
                    for a in range(4):
                        nc.tensor.matmul(
                            jp[:], lhsT=scratch[:, :P], rhs=scratch[:, :nj0],
                            start=(a == 0), stop=(a == 3),
                        )
                # x0 split across the sync+scalar rings (two DMA queues
                # drain concurrently, and phase A's kk=0-3 matmuls only
                # depend on the first half); w1_0 chunked per k-tile so the
                # k-outer matmuls trickle-start chunk by chunk behind it
                h0 = (KT // 2) * nj0
                nc.sync.dma_start(xt[:, :h0], xp[:, :h0])
                nc.scalar.dma_start(
                    xt[:, h0 : KT * nj0], xp[:, h0 : KT * nj0]
                )
                for kk in range(KT):
                    nc.sync.dma_start(w1_0[:, kk, :], g3_0[:, kk, :])
                nc.sync.dma_start(
                    w2_00[:], dn[0].rearrange("p (kk c) -> p kk c", kk=FT)
                )
                for half in range(2):
                    accs = [
                        ps0.tile([P, nj0], f32, tag=f"a{t}", name=f"acc{t}")
                        for t in range(8)
                    ]
                    for kk in range(KT):
                        x_ap = xt[:, kk * nj0 : kk * nj0 + nj0]
                        for i in range(4):
                            ii = half * 4 + i
                            nc.tensor.matmul(
                                accs[2 * i][:],
                                lhsT=w1_0[:, kk, ii * P : (ii + 1) * P],
                                rhs=x_ap, start=(kk == 0), stop=(kk == KT - 1),
                            )
                            nc.tensor.matmul(
                                accs[2 * i + 1][:],
                                lhsT=w1_0[:, kk, dff + ii * P : dff + (ii + 1) * P],
                                rhs=x_ap, start=(kk == 0), stop=(kk == KT - 1),
                            )
                    for i in range(4):
                        ii = half * 4 + i
                        silu_sb = evict.tile([P, nj0], f32, tag="silu")
                        nc.scalar.activation(
                            silu_sb[:], accs[2 * i][:],
                            mybir.ActivationFunctionType.Silu, scale=1.0 / 64,
                        )
                        nc.vector.tensor_mul(
                            act_0[:, ii, :nj0], silu_sb[:], accs[2 * i + 1][:]
                        )

            with (
                tc.tile_pool(name="ps1", bufs=2, space="PSUM") as ps1,
                tc.tile_pool(name="ps2", bufs=2, space="PSUM") as ps2,
            ):
                def mm1_swiglu(w1, act, base, nj, i):
                    gate_ps = ps1.tile([P, nj], f32, tag="gate")
                    up_ps = ps1.tile([P, nj], f32, tag="up")
                    for kk in range(KT):
                        nc.tensor.matmul(
                            gate_ps[:],
                            lhsT=w1[:, kk, i * P : (i + 1) * P],
                            rhs=xt[:, base + kk * nj : base + kk * nj + nj],
                            start=(kk == 0),
                            stop=(kk == KT - 1),
                        )
                    for kk in range(KT):
                        nc.tensor.matmul(
                            up_ps[:],
                            lhsT=w1[:, kk, dff + i * P : dff + (i + 1) * P],
                            rhs=xt[:, base + kk * nj : base + kk * nj + nj],
                            start=(kk == 0),
                            stop=(kk == KT - 1),
                        )
                    silu_sb = evict.tile([P, nj], f32, tag="silu")
                    nc.scalar.activation(
                        silu_sb[:], gate_ps[:],
                        mybir.ActivationFunctionType.Silu, scale=1.0 / 64,
                    )
                    nc.vector.tensor_mul(act[:, i, :nj], silu_sb[:], up_ps[:])

                def mm2_store(w2, act, j, nj):
                    yt = ypool.tile([P, OT * nj], bf, tag="yt")
                    ybase = OT * offs[j]
                    last = j == epc - 1
                    for oi in range(OT):
                        y_ps = ps2.tile([P, nj], f32, tag="y")
                        for kk in range(FT):
                            nc.tensor.matmul(
                                y_ps[:],
                                lhsT=w2[:, kk, oi * P : (oi + 1) * P],
                                rhs=act[:, kk, :nj],
                                start=(kk == 0),
                                stop=(kk == FT - 1),
                            )
                        nc.vector.tensor_copy(
                            yt[:, oi * nj : (oi + 1) * nj], y_ps[:]
                        )
                        if last and (oi % 2 == 1 or oi >= OT - 2):
                            # stream the tail out per oi-pair (singles for
                            # the final two): the last chunk's wire+sem
                            # latency gates the framework teardown
                            lo = oi - 1 if (oi % 2 == 1 and oi < OT - 2) else oi
                            nc.scalar.dma_start(
                                y[:, ybase + lo * nj : ybase + (oi + 1) * nj],
                                yt[:, lo * nj : (oi + 1) * nj],
                            )
                    if not last:
                        # scalar ring so stores never delay the weight stream
                        nc.scalar.dma_start(y[:, ybase : ybase + OT * nj], yt[:])

                # Two overlap tricks at each expert boundary:
                # * expert j's first two mm1 i-blocks are emitted BEFORE
                #   expert j-1's mm2, so the PE fills the window where
                #   act_{j-1}'s last silu is still in flight
                # * w2_{j-1}'s DMA is issued AFTER x_j/w1_j on the ring:
                #   mm1_j[i0] needs w1_j a few us before mm2_{j-1} needs
                #   w2_{j-1}, so the wire delivers in consumption order
                mm2_store(w2_00, act_0, 0, nj0)
                act_prev = j_prev = nj_prev = None
                for j in range(1, epc):
                    nj = cnts[j]
                    base = KT * offs[j]
                    nc.sync.dma_start(
                        xt[:, base : base + KT * nj],
                        xp[:, base : base + KT * nj],
                    )
                    w1 = w1pool.tile([P, KT, 2 * dff], fp8, tag="w1")
                    nc.sync.dma_start(
                        w1[:], gup[j].rearrange("p (kk c) -> p kk c", kk=KT)
                    )
                    if j > 1:
                        w2p = w2pool.tile([P, FT, d], bf, tag="w2")
                        nc.sync.dma_start(
                            w2p[:],
                            dn[j_prev].rearrange("p (kk c) -> p kk c", kk=FT),
                        )
                    act = actpool.tile([P, FT, nmax], bf, tag="act")
                    if j > 1:
                        for i in range(2):
                            mm1_swiglu(w1, act, base, nj, i)
                        mm2_store(w2p, act_prev, j_prev, nj_prev)
                        for i in range(2, FT):
                            mm1_swiglu(w1, act, base, nj, i)
                    else:
                        for i in range(FT):
                            mm1_swiglu(w1, act, base, nj, i)
                    act_prev, j_prev, nj_prev = act, j, nj
                w2l = w2pool.tile([P, FT, d], bf, tag="w2")
                nc.sync.dma_start(
                    w2l[:], dn[j_prev].rearrange("p (kk c) -> p kk c", kk=FT)
                )
                mm2_store(w2l, act_prev, j_prev, nj_prev)

    nc.compile()
    return nc


def _route(topk_indices: np.ndarray, E: int, C: int):
    """Reference-equivalent routing: per expanded token, its within-expert
    position in flat (t, k) order; tokens beyond capacity C are dropped."""
    e = np.asarray(topk_indices).reshape(-1).astype(np.int64)
    N = e.shape[0]
    order = np.argsort(e, kind="stable")  # grouped by expert, flat order kept
    counts = np.bincount(e, minlength=E)
    starts = np.zeros(E + 1, np.int64)
    np.cumsum(counts, out=starts[1:])
    rank = np.arange(N, dtype=np.int64) - starts[e[order]]  # pos within expert
    pos = np.empty(N, np.int64)
    pos[order] = rank
    keep = pos < C
    return e, pos, keep, counts


def kernel(
    hidden_states: np.ndarray,
    topk_indices: np.ndarray,
    topk_weights: np.ndarray,
    gate_up_proj: np.ndarray,
    down_proj: np.ndarray,
) -> np.ndarray:
    hs = np.asarray(hidden_states, dtype=np.float32)
    tw = np.asarray(topk_weights, dtype=np.float32)
    gupw = np.asarray(gate_up_proj, dtype=np.float32)
    dnw = np.asarray(down_proj, dtype=np.float32)

    T, d = hs.shape
    k = np.asarray(topk_indices).shape[-1]
    E, _, dff2 = gupw.shape
    dff = dff2 // 2
    N = T * k
    C = (2 * N) // E  # CAPACITY_FACTOR = 2
    epc = E // NCORES
    KT = d // P
    OT = d // P

    e, pos, keep, _ = _route(topk_indices, E, C)
    posc = np.minimum(pos, C - 1)
    kept_idx = np.where(keep)[0]

    # Kept-token count per expert, then deal experts to (slot, core) by global
    # rank: slot j on every core holds the experts ranked 8j..8j+7 by count.
    # Per-slot counts are then nearly equal across cores, so the SPMD padding
    # (max over cores) wastes ~2% instead of ~10%. Slot 0 is the biggest
    # (overlaps the startup DMA trickle), the last slot smallest (short tail).
    ce = np.bincount(e[kept_idx], minlength=E)
    order = np.argsort(-ce, kind="stable")  # experts by count, descending
    assign = order.reshape(epc, NCORES)  # [slot j, core m] -> expert id
    core_of_expert = np.empty(E, np.int64)
    slot_of_expert = np.empty(E, np.int64)
    for j in range(epc):
        for m in range(NCORES):
            core_of_expert[assign[j, m]] = m
            slot_of_expert[assign[j, m]] = j

    cnts = tuple(
        int(max(int(ce[assign[j]].max()), 1)) for j in range(epc)
    )
    ctot = sum(cnts)
    offs_prog = np.zeros(epc, np.int64)
    np.cumsum(np.asarray(cnts[:-1], np.int64), out=offs_prog[1:])

    core_of = core_of_expert[e]  # per expanded token
    # column of each kept expanded token inside its core's packed layout
    col = offs_prog[slot_of_expert[e]] + posc  # valid where keep

    key = (cnts, d, dff, epc)
    nc = _program_cache.get(key)
    if nc is None:
        nc = _build_program(cnts, d, dff, epc)
        _program_cache[key] = nc

    hsb = hs.astype(BF16)
    tok_of_n = np.arange(N, dtype=np.int64) // k

    # Weights partition-major: [P, KT*cols] with per-partition contiguous
    # bytes (row d-index kk*128+p -> [p, kk*cols:(kk+1)*cols]).
    gup_r = np.ascontiguousarray(
        (gupw * 64.0).reshape(E, KT, P, 2 * dff).transpose(0, 2, 1, 3)
    ).reshape(E, P, KT * 2 * dff).astype(ml_dtypes.float8_e3m4)
    dn_r = np.ascontiguousarray(
        dnw.reshape(E, dff // P, P, d).transpose(0, 2, 1, 3)
    ).reshape(E, P, (dff // P) * d).astype(BF16)

    in_maps = []
    for m in range(NCORES):
        X = np.zeros((ctot, d), BF16)
        sel = kept_idx[core_of[kept_idx] == m]
        X[col[sel]] = hsb[tok_of_n[sel]]
        xT4 = X.T.reshape(KT, P, ctot)  # [kk, p, col]
        xp = np.empty((P, KT * ctot), BF16)
        for j in range(epc):
            off, nj = int(offs_prog[j]), cnts[j]
            xp[:, KT * off : KT * (off + nj)] = (
                xT4[:, :, off : off + nj].transpose(1, 0, 2).reshape(P, KT * nj)
            )
        eids = assign[:, m]  # this core's experts in program (slot) order
        in_maps.append(
            {
                "xp": xp,
                "gup": np.ascontiguousarray(gup_r[eids]),
                "dn": np.ascontiguousarray(dn_r[eids]),
            }
        )

    res = run_bass_kernel_spmd(nc, in_maps, list(range(NCORES)))

    # combine: rows[n] = y_core(n)[:, col(n)] for kept n, 0 otherwise
    rows = np.zeros((N, d), np.float32)
    for m in range(NCORES):
        Yp = np.asarray(res.results[m]["y"])  # [P, OT*ctot] bf16 packed
        Y = np.empty((d, ctot), np.float32)
        for j in range(epc):
            off, nj = int(offs_prog[j]), cnts[j]
            Y[:, off : off + nj] = (
                Yp[:, OT * off : OT * (off + nj)]
                .reshape(P, OT, nj).transpose(1, 0, 2).reshape(d, nj)
            )
        sel = kept_idx[core_of[kept_idx] == m]
        rows[sel] = Y.T[col[sel]]
    wf = tw.reshape(-1) * keep.astype(np.float32) * (1.0 / 64.0)
    out = (rows * wf[:, None]).reshape(T, k, d).sum(axis=1)
    return out.astype(hs.dtype)
